# revision 1
# baseline (speedup 1.0000x reference)
"""Trainium2 kernel: X = inv(phi + sigma2*A) for the DeepKernelPacketGP module.

Host (f64, O(n) prep): pentadiagonal bands of B via batched 5x5 kernel-packet
window solves; boundary Riccati scans; dressed leaf inverses; per-tree-node
2x2 chain coefficients + dressed interface strips.
Device (fp32, O(n^2) work, 8 cores, column-slab sharding): log-depth boundary
-row chains down a bisection tree + all leaf row-block writes via PE matmuls;
each core materializes X[:, core*512:(core+1)*512].
"""
import sys
sys.path.insert(0, '/opt/trn_rl_repo')
import numpy as np

N = 4096
NB = 64                    # leaf span size
NLEAF = N // NB            # 64
LEVELS = 6                 # 2^6 leaves
NCORES = 8
SLAB = N // NCORES         # 512

# ============================================================================
# Host math (float64)
# ============================================================================

def _stage1_bands(x, rho, sigma2):
    n = x.shape[0]; k = 5; m = 2; n_pow = 2
    c = np.sqrt(3.0) / rho
    W = n - 4
    idx = np.arange(W)[:, None] + np.arange(k)[None, :]
    xw = x[idx]
    t = xw - (xw[:, :1] + xw[:, -1:]) / 2
    pw = t[:, :, None] ** np.arange(n_pow)
    pos = pw * np.exp(c * t)[:, :, None]
    neg = pw * np.exp(-c * t)[:, :, None]
    e_first = np.zeros((W, 1, k)); e_first[:, :, 0] = 1.0
    Amat = np.concatenate([np.swapaxes(pos, 1, 2), np.swapaxes(neg, 1, 2), e_first], axis=1)
    rhs = np.zeros((k,)); rhs[-1] = 1.0
    a = np.linalg.solve(Amat, np.broadcast_to(rhs, (W, k))[..., None])[..., 0]
    d = np.abs(xw[:, :, None] - xw[:, None, :]); s = c * d
    Kw = (1 + s) * np.exp(-s)
    phiv = np.einsum('wij,wj->wi', Kw, a)
    bcol = phiv + sigma2 * a
    Bcols = np.zeros((n, 5))
    Bcols[2:n-2, :] = bcol
    def bnd(xseg, tshift, npos, nneg):
        ss = xseg.shape[0]
        xt = xseg + tshift
        rows = [xt**j * np.exp(c*xt) for j in range(npos)]
        rows += [xt**j * np.exp(-c*xt) for j in range(nneg)]
        e = np.zeros(ss); e[0] = 1.0
        rows.append(e)
        M = np.stack(rows); r = np.zeros(ss); r[-1] = 1.0
        aa = np.linalg.solve(M, r)
        dd = np.abs(xseg[:, None] - xseg[None, :]); s2 = c*dd
        return aa, ((1+s2)*np.exp(-s2)) @ aa
    for i in range(m):
        s_l = i + m + 1
        aa, pp = bnd(x[:s_l], -x[s_l-1], n_pow, s_l - 3)
        for r in range(s_l):
            Bcols[i, r - i + 2] = pp[r] + sigma2*aa[r]
        s_r = k - 1 - i
        aa, pp = bnd(x[n-s_r:], -x[n-s_r], s_r - 3, n_pow)
        col = n - m + i
        for ridx in range(s_r):
            r = n - s_r + ridx
            Bcols[col, r - col + 2] = pp[ridx] + sigma2*aa[ridx]
    return Bcols


def _bands_by_diag(Bcols):
    n = Bcols.shape[0]
    bd = {d: np.zeros(n) for d in range(-2, 3)}
    for j in range(5):
        c0 = max(0, 2 - j); c1 = min(n, n + 2 - j)
        for col in range(c0, c1):
            r = col - 2 + j
            bd[col - r][r] = Bcols[col, j]
    return bd


def _span_matrix(bd, lo, hi):
    s = hi - lo
    M = np.zeros((s, s))
    for d in range(-2, 3):
        r0 = max(0, -d); r1 = min(s, s - d)
        rr = np.arange(r0, r1)
        M[rr, rr + d] = bd[d][lo + rr]
    return M


def _c_coup(bd, b):
    return np.array([[bd[2][b-2], 0.0], [bd[1][b-1], bd[2][b-1]]])


def _e_coup(bd, b):
    return np.array([[bd[-2][b], bd[-1][b]], [0.0, bd[-2][b+1]]])


def _banded_solve(bd, lo, hi, tl, br, rhs, transpose=False):
    """Solve (B_span - mods) X = rhs (dense np solve for simplicity on small
    spans; LU via scipy-free banded elimination for big spans)."""
    D = _span_matrix(bd, lo, hi)
    if tl is not None: D[:2, :2] -= tl
    if br is not None: D[-2:, -2:] -= br
    if transpose:
        D = D.T
    return np.linalg.solve(D, rhs)


def _host_pieces(bd):
    n = N; nl = NLEAF
    GL = np.zeros((nl+1, 2, 2))
    for k in range(1, nl+1):
        lo = (k-1)*NB
        D = _span_matrix(bd, lo, lo+NB)
        if k > 1:
            D[:2, :2] -= _e_coup(bd, lo) @ GL[k-1] @ _c_coup(bd, lo)
        GL[k] = np.linalg.inv(D)[-2:, -2:]
    GR = np.zeros((nl+1, 2, 2))
    for k in range(nl-1, -1, -1):
        lo = k*NB
        D = _span_matrix(bd, lo, lo+NB)
        if k < nl-1:
            b = lo + NB
            D[-2:, -2:] -= _c_coup(bd, b) @ GR[k+1] @ _e_coup(bd, b)
        GR[k] = np.linalg.inv(D)[:2, :2]
    Xhat = np.zeros((nl, NB, NB))
    gTLe = np.zeros((nl, NB, 2))
    gBRe = np.zeros((nl, NB, 2))
    for ell in range(nl):
        lo = ell*NB; hi = lo + NB
        D0 = _span_matrix(bd, lo, hi)
        TLm = np.zeros((NB, NB)); BRm = np.zeros((NB, NB))
        if lo > 0:
            TLm[:2, :2] = _e_coup(bd, lo) @ GL[ell] @ _c_coup(bd, lo)
        if hi < n:
            BRm[-2:, -2:] = _c_coup(bd, hi) @ GR[ell+1] @ _e_coup(bd, hi)
        Xhat[ell] = np.linalg.inv(D0 - TLm - BRm)
        if hi < n:
            gTLe[ell] = -np.linalg.inv(D0 - TLm)[:, -2:] @ _c_coup(bd, hi)
        if lo > 0:
            gBRe[ell] = -np.linalg.inv(D0 - BRm)[:, :2] @ _e_coup(bd, lo)

    def dressed_rows(lo, hi, tl, br, rows):
        s = hi - lo
        rhs = np.zeros((s, len(rows)))
        for i, r in enumerate(rows):
            rhs[r, i] = 1.0
        return _banded_solve(bd, lo, hi, tl, br, rhs, transpose=True).T

    nodes = []
    for L in range(1, LEVELS+1):
        sz = NB * 2**L
        cnt = n // sz
        CA = np.zeros((cnt, 2, 2)); DA = np.zeros((cnt, 2, 2))
        CB = np.zeros((cnt, 2, 2)); DB = np.zeros((cnt, 2, 2))
        sA = np.zeros((cnt, 2, sz//2)); sB = np.zeros((cnt, 2, sz//2))
        for i in range(cnt):
            mlo = i*sz; mhi = mlo + sz; mid = mlo + sz//2
            kA0 = mlo//NB; kA1 = mid//NB; kB1 = mhi//NB
            tlA = brB = None
            if mlo > 0:
                tlA = _e_coup(bd, mlo) @ GL[kA0] @ _c_coup(bd, mlo)
            if mhi < n:
                brB = _c_coup(bd, mhi) @ GR[kB1] @ _e_coup(bd, mhi)
            cM = _c_coup(bd, mid); eM = _e_coup(bd, mid)
            brA = cM @ GR[kA1] @ eM
            tlB = eM @ GL[kA1] @ cM
            half = sz//2
            rA = dressed_rows(mlo, mid, tlA, None, [half-2, half-1])
            CA[i] = -(rA[:, -2:]) @ cM
            rB = dressed_rows(mid, mhi, tlB, None, [0, 1])
            CB[i] = -(rB[:, -2:]) @ (_c_coup(bd, mhi) if mhi < n else np.zeros((2, 2)))
            rA2 = dressed_rows(mlo, mid, None, brA, [half-2, half-1])
            DA[i] = -(rA2[:, :2]) @ (_e_coup(bd, mlo) if mlo > 0 else np.zeros((2, 2)))
            rB2 = dressed_rows(mid, mhi, None, brB, [0, 1])
            DB[i] = -(rB2[:, :2]) @ eM
            sA[i] = dressed_rows(mlo, mid, tlA, brA, [half-2, half-1])
            sB[i] = dressed_rows(mid, mhi, tlB, brB, [0, 1])
        nodes.append(dict(CA=CA, DA=DA, CB=CB, DB=DB, sA=sA, sB=sB, sz=sz))
    return dict(GL=GL, GR=GR, Xhat=Xhat, gTLe=gTLe, gBRe=gBRe, nodes=nodes)


def _core_inputs(P, core):
    col_lo = core * SLAB
    cols = np.arange(col_lo, col_lo + SLAB)
    f32 = np.float32
    ins = {}
    for L in range(LEVELS, 0, -1):
        nd = P["nodes"][L-1]
        sz = nd["sz"]; cnt = N // sz
        coef = np.zeros((cnt, 16), f32)
        coef[:, 0:4] = nd["CA"].reshape(cnt, 4)
        coef[:, 4:8] = nd["DA"].reshape(cnt, 4)
        coef[:, 8:12] = nd["CB"].reshape(cnt, 4)
        coef[:, 12:16] = nd["DB"].reshape(cnt, 4)
        ins[f"coef{L}"] = coef
        strip = np.zeros((cnt, 4, SLAB), f32)
        thr = np.zeros((cnt, 4), f32)
        for i in range(cnt):
            mlo = i*sz; mid = mlo + sz//2; mhi = mlo + sz
            thr[i] = [mlo, mid, mhi, 0]
            mA = (cols >= mlo) & (cols < mid)
            mB = (cols >= mid) & (cols < mhi)
            if mA.any():
                strip[i, 0:2, mA] = nd["sA"][i][:, cols[mA]-mlo].astype(f32).T
            if mB.any():
                strip[i, 2:4, mB] = nd["sB"][i][:, cols[mB]-mid].astype(f32).T
        ins[f"strip{L}"] = strip.reshape(cnt, 4*SLAB)
        ins[f"thr{L}"] = thr
    # leaf-write matrices: groups of 2 leaves (128 rows); lhsT [8, 128]
    lmatT = np.zeros((32, 8, 128), f32)
    rmask = np.zeros((32, 8, SLAB), f32)
    for g in range(32):
        for li in range(2):
            ell = 2*g + li
            blk = np.zeros((NB, 4))
            blk[:, 0:2] = P["gTLe"][ell]     # multiplies bl rows
            blk[:, 2:4] = P["gBRe"][ell]     # multiplies ab rows
            lmatT[g, li*4:(li+1)*4, li*NB:(li+1)*NB] = blk.T
            lo = ell*NB; hi = lo + NB
            rmask[g, li*4+0:li*4+2, :] = (cols >= hi).astype(f32)[None, :]
            rmask[g, li*4+2:li*4+4, :] = (cols < lo).astype(f32)[None, :]
    ins["lmatT"] = lmatT
    ins["rmask"] = rmask
    # diag inserts: 4 groups per core; [4, 128, 128]
    xh = np.zeros((4, 128, 128), f32)
    for j in range(4):
        g = core*4 + j
        for li in range(2):
            ell = 2*g + li
            xh[j, li*NB:(li+1)*NB, li*NB:(li+1)*NB] = P["Xhat"][ell]
    ins["xhat"] = xh.transpose(1, 0, 2).reshape(128, 4*128).copy()
    ins["colidx"] = np.broadcast_to(cols.astype(f32), (128, SLAB)).copy()
    dfl = np.zeros((128, 32), f32)
    for j in range(4):
        dfl[:, core*4 + j] = 1.0
    ins["diagflag"] = dfl
    return ins


# ============================================================================
# Device kernel
# ============================================================================

_CACHED = {}

def _build_nc():
    import concourse.bass as bass
    import concourse.mybir as mybir
    import concourse.tile as tile
    from concourse.vector_clock import ScopedClock

    def _patched_drain_and_barrier(self, tick_clock, wait_clock):
        nopw = self.nc.gpsimd.nop()
        wait_clock.add_sem_waits(nopw.ins, ScopedClock({None: tick_clock.global_clock}))
        waits = list(nopw.ins.sync_info.on_wait) if nopw.ins.sync_info else []
        if len(waits) > 1:
            nopw.ins.sync_info.on_wait = waits[:1]
            for w in waits[1:]:
                extra = self.nc.gpsimd.nop()
                extra.ins.sync_info = mybir.SyncInfo(on_wait=[w], on_update=[])
        self.nc.sync.drain()
        self.nc.all_engine_barrier()
        assert self.sems is not None
        popped = self.nc._tile_sem_poison_stack.pop()
        assert popped is self._sem_poison
        self.nc.clear_and_free_semaphores(list(self.sems.allocated().values()))
        self.nc.all_engine_barrier()
    tile.TileContext._drain_and_barrier = _patched_drain_and_barrier

    F32 = mybir.dt.float32
    MUL = mybir.AluOpType.mult
    ADD = mybir.AluOpType.add
    GE = mybir.AluOpType.is_ge
    LT = mybir.AluOpType.is_lt
    S = SLAB

    nc = bass.Bass(target_bir_lowering=False)
    dins = {}
    for L in range(LEVELS, 0, -1):
        cnt = N // (NB * 2**L)
        dins[f"coef{L}"] = nc.dram_tensor(f"coef{L}", [cnt, 16], F32, kind="ExternalInput")
        dins[f"strip{L}"] = nc.dram_tensor(f"strip{L}", [cnt, 4*S], F32, kind="ExternalInput")
        dins[f"thr{L}"] = nc.dram_tensor(f"thr{L}", [cnt, 4], F32, kind="ExternalInput")
    dins["lmatT"] = nc.dram_tensor("lmatT", [32, 8, 128], F32, kind="ExternalInput")
    dins["rmask"] = nc.dram_tensor("rmask", [32, 8, S], F32, kind="ExternalInput")
    dins["xhat"] = nc.dram_tensor("xhat", [128, 4*128], F32, kind="ExternalInput")
    dins["colidx"] = nc.dram_tensor("colidx", [128, S], F32, kind="ExternalInput")
    dins["diagflag"] = nc.dram_tensor("diagflag", [128, 32], F32, kind="ExternalInput")
    dout = nc.dram_tensor("xslab", [N, S], F32, kind="ExternalOutput")

    with tile.TileContext(nc) as tc:
        with tc.tile_pool(name="main", bufs=1) as pool, \
             tc.tile_pool(name="io", bufs=2) as iopool, \
             tc.tile_pool(name="ps", bufs=4, space="PSUM") as pspool:
            colidx = pool.tile([128, S], F32, tag="colidx")
            nc.sync.dma_start(colidx[:], dins["colidx"][:])
            # boundary tiles per span-level: bnd_k has (64/2^k spans)+1 rows
            bnd = {}
            for Lspan in range(LEVELS + 1):
                rows = (N // (NB * 2**Lspan)) + 1
                t = pool.tile([rows, 4*S], F32, tag=f"bnd{Lspan}")
                nc.vector.memset(t[:], 0.0)
                bnd[Lspan] = t
            for L in range(LEVELS, 0, -1):
                cnt = N // (NB * 2**L)
                coef = pool.tile([cnt, 16], F32, tag="coef")
                strip = pool.tile([cnt, 4*S], F32, tag="strip")
                thr = pool.tile([cnt, 4], F32, tag="thr")
                nc.sync.dma_start(coef[:], dins[f"coef{L}"][:])
                nc.sync.dma_start(strip[:], dins[f"strip{L}"][:])
                nc.sync.dma_start(thr[:], dins[f"thr{L}"][:])
                prev = bnd[L]           # [cnt+1, 4S] boundaries of level-L spans
                newb = pool.tile([cnt, 4*S], F32, tag="newb")
                tmp = pool.tile([cnt, 2*S], F32, tag="tmpc")
                tmp2 = pool.tile([cnt, 2*S], F32, tag="tmp2c")
                msk = pool.tile([cnt, S], F32, tag="mskc")
                a2 = prev[0:cnt, 0:2*S]          # u-part of left boundary
                b2 = pool.tile([cnt, 2*S], F32, tag="b2t")
                nc.sync.dma_start(b2[:], prev[1:cnt+1, 2*S:4*S])
                b2 = b2[:]
                u = newb[:, 0:2*S]; v = newb[:, 2*S:4*S]

                def mat2_apply(dst, cbase, src):
                    # dst[:,r*S:(r+1)*S] = c[2r]*src_row0 + c[2r+1]*src_row1
                    for r in range(2):
                        nc.vector.tensor_scalar(
                            tmp2[:, r*S:(r+1)*S], src[:, 0:S],
                            coef[:, cbase+2*r:cbase+2*r+1], None, MUL)
                        nc.vector.tensor_scalar(
                            dst[:, r*S:(r+1)*S], src[:, S:2*S],
                            coef[:, cbase+2*r+1:cbase+2*r+2], None, MUL)
                        nc.vector.tensor_tensor(
                            dst[:, r*S:(r+1)*S], dst[:, r*S:(r+1)*S],
                            tmp2[:, r*S:(r+1)*S], ADD)

                def apply_mask(dst, thr_col, op):
                    nc.vector.tensor_scalar(msk[:], colidx[0:cnt, :],
                                            thr[:, thr_col:thr_col+1], None, op)
                    for r in range(2):
                        nc.vector.tensor_tensor(dst[:, r*S:(r+1)*S],
                                                dst[:, r*S:(r+1)*S], msk[:], MUL)

                # u_a = (DA @ a2)*[col < mlo] + stripA
                mat2_apply(u, 4, a2)
                apply_mask(u, 0, LT)
                nc.vector.tensor_tensor(u, u, strip[:, 0:2*S], ADD)
                # v = (DB @ u_a)*[col < mid] + stripB + (CB @ b2)*[col >= mhi]
                mat2_apply(v, 12, u)
                apply_mask(v, 1, LT)
                nc.vector.tensor_tensor(v, v, strip[:, 2*S:4*S], ADD)
                mat2_apply(tmp, 8, b2)
                apply_mask(tmp, 2, GE)
                nc.vector.tensor_tensor(v, v, tmp[:], ADD)
                # u += (CA @ v)*[col >= mid]
                mat2_apply(tmp, 0, v)
                apply_mask(tmp, 1, GE)
                nc.vector.tensor_tensor(u, u, tmp[:], ADD)
                # interleave into bnd[L-1]: even <- prev, odd <- newb
                nxt = bnd[L-1]
                import concourse.bass as _b
                nc.sync.dma_start(
                    _b.AP(nxt.tensor, nxt.offset, [[2*(4*S), cnt+1], [1, 4*S]]),
                    prev[0:cnt+1, :])
                nc.sync.dma_start(
                    _b.AP(nxt.tensor, nxt.offset + 4*S, [[2*(4*S), cnt], [1, 4*S]]),
                    newb[:, :])
            bleaf = bnd[0]   # [65, 4S]
            # ---- leaf writes ----
            import concourse.bass as _b
            xh = pool.tile([128, 4*128], F32, tag="xh")
            nc.sync.dma_start(xh[:], dins["xhat"][:])
            dfl = pool.tile([128, 32], F32, tag="dfl")
            nc.sync.dma_start(dfl[:], dins["diagflag"][:])
            # R-all [8, 32*S]: row p=li*4+q (li=leaf in group, q=0..3):
            #   q=0,1: bl rows of leaf (v-part rows q of boundary 2g+li+1)
            #   q=2,3: ab rows (u-part rows q-2 of boundary 2g+li)
            Rall = pool.tile([8, 32*S], F32, tag="Rall")
            bl_ap = bleaf[:]
            fsz = 4*S
            for li in range(2):
                for q in range(4):
                    p = li*4 + q
                    if q < 2:
                        # src partition 2g+li+1, free offset (2+q)*S
                        srcoff = (li+1)*fsz + (2+q)*S
                    else:
                        srcoff = li*fsz + (q-2)*S
                    nc.sync.dma_start(
                        _b.AP(Rall[:].tensor, Rall[:].offset + p*(32*S),
                              [[32*S, 1], [S, 32], [1, S]]),
                        _b.AP(bl_ap.tensor, bl_ap.offset + srcoff,
                              [[2*fsz, 32], [1, S]]))
            for g in range(32):
                lm = iopool.tile([8, 128], F32, tag="lm")
                nc.sync.dma_start(lm[:], dins["lmatT"][g])
                rm = iopool.tile([8, S], F32, tag="rm")
                nc.sync.dma_start(rm[:], dins["rmask"][g])
                nc.vector.tensor_tensor(Rall[:, g*S:(g+1)*S], Rall[:, g*S:(g+1)*S], rm[:], MUL)
                ps = pspool.tile([128, S], F32, tag="ps")
                nc.tensor.matmul(ps[:], lm[:], Rall[:, g*S:(g+1)*S])
                ob = iopool.tile([128, S], F32, tag="ob")
                nc.scalar.copy(ob[:], ps[:])
                j = g % 4
                tmpd = iopool.tile([128, 128], F32, tag="tmpd")
                nc.vector.tensor_scalar(tmpd[:], xh[:, j*128:(j+1)*128],
                                        dfl[:, g:g+1], None, MUL)
                nc.vector.tensor_tensor(ob[:, j*128:(j+1)*128],
                                        ob[:, j*128:(j+1)*128], tmpd[:], ADD)
                nc.sync.dma_start(dout[g*128:(g+1)*128, :], ob[:])
    # --- post-pass: this walrus build allows only 1 sync-wait per
    # instruction; split extras onto preceding same-engine NOPs ---
    def _split_waits(maxw=1):
        all_bbs = list(nc.main_func.blocks)
        for bb in all_bbs:
            out = []
            for inst in bb.instructions:
                si = getattr(inst, "sync_info", None)
                ow = list(si.on_wait) if (si is not None and si.on_wait) else []
                if len(ow) > maxw:
                    si.on_wait = ow[-maxw:]
                    try:
                        eng_builder = nc.engines[inst.engine]
                    except Exception:
                        eng_builder = nc.sync
                    for w in ow[:-maxw]:
                        nop = eng_builder.nop()
                        for bb2 in nc.main_func.blocks:
                            li = bb2.instructions
                            if li and li[-1] is nop.ins:
                                li.pop()
                                break
                        nop.ins.sync_info = mybir.SyncInfo(on_wait=[w], on_update=[])
                        out.append(nop.ins)
                out.append(inst)
            bb.instructions[:] = out
    _split_waits()
    return nc, dins, dout


def _device_run(P, timeit=False):
    from concourse.bass_utils import run_bass_kernel_spmd
    if "nc" not in _CACHED:
        _CACHED["nc"] = _build_nc()
    nc, dins, dout = _CACHED["nc"]
    in_maps = [_core_inputs(P, core) for core in range(NCORES)]
    res = run_bass_kernel_spmd(nc, in_maps, list(range(NCORES)))
    slabs = [res.results[c]["xslab"] for c in range(NCORES)]
    return np.concatenate(slabs, axis=1)


def kernel(x, rho, sigma2):
    x = np.asarray(x, dtype=np.float64)
    rho = float(np.asarray(rho)); sigma2 = float(np.asarray(sigma2))
    Bcols = _stage1_bands(x, rho, sigma2)
    bd = _bands_by_diag(Bcols)
    P = _host_pieces(bd)
    _CACHED["P_obj"] = P
    X = _device_run(P).astype(np.float64)
    return X



# revision 5
# speedup vs baseline: 4.7775x; 4.7775x over previous
"""Trainium2 kernel: X = inv(phi + sigma2*A) for the DeepKernelPacketGP module.

Math: B = phi + sigma2*A is exactly pentadiagonal, so X = inv(B) is
semiseparable: X[i,j] = F[i,:] @ C[:,j] for i < j (rank 2), G[i,:] @ D[:,j]
for i > j, plus the diagonal. Host (f64, O(n)): banded solves for F, G
(columns 0,1 and n-2,n-1 of X), batched local 5x5 solves per column for
C, D, diag. Device (8 cores, column-slab sharding): per 128x512 output
tile one K=4 fp32r matmul (per-row-block QR-orthonormalized generators),
PSUM->SBUF copy, DMA out. Diagonal 128x128 blocks are host-precomputed in
f64 and overwritten after the matmul; each core processes its 4 diagonal
tiles first so the overwrite offset is core-invariant (SPMD), and the host
unscrambles the row order on gather.
"""
import sys
sys.path.insert(0, '/opt/trn_rl_repo')
import numpy as np
from scipy.linalg import solve_banded

N = 4096
NCORES = 8
SLAB = N // NCORES         # 512
TB = 128                   # tile rows
NT = N // TB               # 32 tiles per slab

# ============================================================================
# Host math (float64)
# ============================================================================

def _stage1_bands(x, rho, sigma2):
    n = x.shape[0]; k = 5; m = 2; n_pow = 2
    c = np.sqrt(3.0) / rho
    W = n - 4
    idx = np.arange(W)[:, None] + np.arange(k)[None, :]
    xw = x[idx]
    t = xw - (xw[:, :1] + xw[:, -1:]) / 2
    pw = t[:, :, None] ** np.arange(n_pow)
    pos = pw * np.exp(c * t)[:, :, None]
    neg = pw * np.exp(-c * t)[:, :, None]
    e_first = np.zeros((W, 1, k)); e_first[:, :, 0] = 1.0
    Amat = np.concatenate([np.swapaxes(pos, 1, 2), np.swapaxes(neg, 1, 2), e_first], axis=1)
    rhs = np.zeros((k,)); rhs[-1] = 1.0
    a = np.linalg.solve(Amat, np.broadcast_to(rhs, (W, k))[..., None])[..., 0]
    d = np.abs(xw[:, :, None] - xw[:, None, :]); s = c * d
    Kw = (1 + s) * np.exp(-s)
    phiv = np.einsum('wij,wj->wi', Kw, a)
    bcol = phiv + sigma2 * a
    Bcols = np.zeros((n, 5))
    Bcols[2:n-2, :] = bcol
    def bnd(xseg, tshift, npos, nneg):
        ss = xseg.shape[0]
        xt = xseg + tshift
        rows = [xt**j * np.exp(c*xt) for j in range(npos)]
        rows += [xt**j * np.exp(-c*xt) for j in range(nneg)]
        e = np.zeros(ss); e[0] = 1.0
        rows.append(e)
        M = np.stack(rows); r = np.zeros(ss); r[-1] = 1.0
        aa = np.linalg.solve(M, r)
        dd = np.abs(xseg[:, None] - xseg[None, :]); s2 = c*dd
        return aa, ((1+s2)*np.exp(-s2)) @ aa
    for i in range(m):
        s_l = i + m + 1
        aa, pp = bnd(x[:s_l], -x[s_l-1], n_pow, s_l - 3)
        for r in range(s_l):
            Bcols[i, r - i + 2] = pp[r] + sigma2*aa[r]
        s_r = k - 1 - i
        aa, pp = bnd(x[n-s_r:], -x[n-s_r], s_r - 3, n_pow)
        col = n - m + i
        for ridx in range(s_r):
            r = n - s_r + ridx
            Bcols[col, r - col + 2] = pp[ridx] + sigma2*aa[ridx]
    return Bcols


def _host_generators(Bcols):
    """F, G (n x 2), C, D (2 x n), xd (n): semiseparable generators of inv(B)."""
    n = N
    ab = Bcols.T.copy()              # scipy banded form: ab[2+d, c] = B[c+d, c]
    E = np.zeros((n, 4))
    E[n-2, 0] = 1; E[n-1, 1] = 1; E[0, 2] = 1; E[1, 3] = 1
    sol = solve_banded((2, 2), ab, E)
    F = sol[:, 0:2]; G = sol[:, 2:4]

    Brow = np.zeros((5, n))          # Brow[2+d, j] = B[j, j+d]
    for d in range(-2, 3):
        jj = np.arange(max(0, -d), n - max(0, d))
        Brow[2 + d, jj] = Bcols[jj + d, 2 - d]

    def BofRI(r, i):
        d = i - r
        out = np.zeros(r.shape)
        ok = (np.abs(d) <= 2) & (r >= 0) & (r < n) & (i >= 0) & (i < n)
        out[ok] = Brow[2 + d[ok], r[ok]]
        return out

    # interior columns j=2..n-3: 5x5 solve for [c1, c2, xjj, d1, d2]
    jj = np.arange(2, n - 2)
    M = np.zeros((jj.size, 5, 5))
    for r_loc in range(5):
        r = jj - 2 + r_loc
        for t in range(-2, 3):
            i = r + t
            b = BofRI(r, i)
            ic = np.clip(i, 0, n - 1)
            lo = i < jj; eq = i == jj; hi = i > jj
            M[lo, r_loc, 0] += b[lo] * F[ic[lo], 0]
            M[lo, r_loc, 1] += b[lo] * F[ic[lo], 1]
            M[eq, r_loc, 2] += b[eq]
            M[hi, r_loc, 3] += b[hi] * G[ic[hi], 0]
            M[hi, r_loc, 4] += b[hi] * G[ic[hi], 1]
    rhs = np.zeros((jj.size, 5)); rhs[:, 2] = 1.0
    U = np.linalg.solve(M, rhs[..., None])[..., 0]
    C = np.zeros((2, n)); D = np.zeros((2, n)); xd = np.zeros(n)
    C[:, jj] = U[:, 0:2].T
    xd[jj] = U[:, 2]
    D[:, jj] = U[:, 3:5].T

    # boundary columns
    M3 = np.zeros((3, 3)); r3 = np.arange(3)
    for t in range(-2, 3):
        i = r3 + t; b = BofRI(r3, i); ic = np.clip(i, 0, n - 1)
        M3[:, 0] += b * (i == 0)
        for k in range(2):
            M3[:, 1 + k] += b * G[ic, k] * (i > 0)
    u = np.linalg.solve(M3, np.eye(3)[0])
    xd[0] = u[0]; D[:, 0] = u[1:3]

    M4 = np.zeros((4, 4)); r4 = np.arange(4)
    for t in range(-2, 3):
        i = r4 + t; b = BofRI(r4, i); ic = np.clip(i, 0, n - 1)
        M4[:, 0] += b * (i == 0)
        M4[:, 1] += b * (i == 1)
        for k in range(2):
            M4[:, 2 + k] += b * G[ic, k] * (i > 1)
    u = np.linalg.solve(M4, np.eye(4)[1])
    xd[1] = u[1]; D[:, 1] = u[2:4]
    # X[0,1] needed for the (0,0) diagonal center block
    x01 = u[0]

    M4 = np.zeros((4, 4)); r4 = np.arange(n - 4, n)
    for t in range(-2, 3):
        i = r4 + t; b = BofRI(r4, i); ic = np.clip(i, 0, n - 1)
        for k in range(2):
            M4[:, k] += b * F[ic, k] * (i < n - 2)
        M4[:, 2] += b * (i == n - 2)
        M4[:, 3] += b * (i == n - 1)
    u = np.linalg.solve(M4, np.eye(4)[2])
    C[:, n - 2] = u[0:2]; xd[n - 2] = u[2]
    xn12 = u[3]   # X[n-1, n-2]

    M3 = np.zeros((3, 3)); r3 = np.arange(n - 3, n)
    for t in range(-2, 3):
        i = r3 + t; b = BofRI(r3, i); ic = np.clip(i, 0, n - 1)
        for k in range(2):
            M3[:, k] += b * F[ic, k] * (i < n - 1)
        M3[:, 2] += b * (i == n - 1)
    u = np.linalg.solve(M3, np.eye(3)[2])
    C[:, n - 1] = u[0:2]; xd[n - 1] = u[2]

    return F, G, C, D, xd, x01, xn12


def _host_pieces(Bcols):
    F, G, C, D, xd, x01, xn12 = _host_generators(Bcols)
    n = N
    # per-row-block QR bases
    Qf = np.zeros((NT, TB, 2)); Rf = np.zeros((NT, 2, 2))
    Qg = np.zeros((NT, TB, 2)); Rg = np.zeros((NT, 2, 2))
    for g in range(NT):
        rows = slice(TB * g, TB * (g + 1))
        Qf[g], Rf[g] = np.linalg.qr(F[rows])
        Qg[g], Rg[g] = np.linalg.qr(G[rows])
    # diagonal center blocks (f64)
    cen = np.zeros((NT, TB, TB))
    ii = np.arange(TB)[:, None]; jc = np.arange(TB)[None, :]
    for g in range(NT):
        rows = slice(TB * g, TB * (g + 1))
        up = F[rows] @ C[:, rows]
        loP = G[rows] @ D[:, rows]
        blk = np.where(ii < jc, up, np.where(ii > jc, loP, 0.0))
        blk[np.arange(TB), np.arange(TB)] = xd[TB * g:TB * (g + 1)]
        cen[g] = blk
    # exact corner values not covered by the rank-2 reps
    cen[0][0, 1] = x01
    cen[NT - 1][TB - 1, TB - 2] = xn12
    return dict(F=F, G=G, C=C, D=D, xd=xd, Qf=Qf, Rf=Rf, Qg=Qg, Rg=Rg, cen=cen)


def _tile_order(core):
    """Processing order of row-blocks for this core: its 4 diag blocks first."""
    own = [4 * core + k for k in range(4)]
    rest = [g for g in range(NT) if g // 4 != core]
    return own + rest


def _core_inputs(P, core):
    f32 = np.float32
    cols = slice(SLAB * core, SLAB * (core + 1))
    order = _tile_order(core)
    lhsT = np.zeros((4, NT * TB), f32)
    rhs = np.zeros((4, NT * SLAB), f32)
    colblk = (np.arange(SLAB * core, SLAB * (core + 1)) // TB)  # global 128-block
    rhsC_all = {}
    for t, g in enumerate(order):
        lhsT[0:2, t*TB:(t+1)*TB] = P["Qf"][g].T
        lhsT[2:4, t*TB:(t+1)*TB] = P["Qg"][g].T
        mC = (colblk > g).astype(np.float64)
        mD = (colblk < g).astype(np.float64)
        rhs[0:2, t*SLAB:(t+1)*SLAB] = (P["Rf"][g] @ P["C"][:, cols]) * mC
        rhs[2:4, t*SLAB:(t+1)*SLAB] = (P["Rg"][g] @ P["D"][:, cols]) * mD
    diag = np.zeros((TB, 4 * TB), f32)
    for k in range(4):
        diag[:, k*TB:(k+1)*TB] = P["cen"][4 * core + k]
    return {"lhsT": lhsT, "rhs": rhs, "diag": diag}


# ============================================================================
# Device kernel
# ============================================================================

_CACHED = {}

def _build_nc():
    import concourse.bass as bass
    import concourse.mybir as mybir
    import concourse.tile as tile
    from concourse.vector_clock import ScopedClock

    def _patched_drain_and_barrier(self, tick_clock, wait_clock):
        nopw = self.nc.gpsimd.nop()
        wait_clock.add_sem_waits(nopw.ins, ScopedClock({None: tick_clock.global_clock}))
        waits = list(nopw.ins.sync_info.on_wait) if nopw.ins.sync_info else []
        if len(waits) > 1:
            nopw.ins.sync_info.on_wait = waits[:1]
            for w in waits[1:]:
                extra = self.nc.gpsimd.nop()
                extra.ins.sync_info = mybir.SyncInfo(on_wait=[w], on_update=[])
        self.nc.sync.drain()
        self.nc.all_engine_barrier()
        assert self.sems is not None
        popped = self.nc._tile_sem_poison_stack.pop()
        assert popped is self._sem_poison
        self.nc.clear_and_free_semaphores(list(self.sems.allocated().values()))
        self.nc.all_engine_barrier()
    tile.TileContext._drain_and_barrier = _patched_drain_and_barrier

    F32 = mybir.dt.float32
    F32R = mybir.dt.float32r
    S = SLAB

    nc = bass.Bass(target_bir_lowering=False)
    dins = {
        "lhsT": nc.dram_tensor("lhsT", [4, NT * TB], F32R, kind="ExternalInput"),
        "rhs": nc.dram_tensor("rhs", [4, NT * S], F32R, kind="ExternalInput"),
        "diag": nc.dram_tensor("diag", [TB, 4 * TB], F32, kind="ExternalInput"),
    }
    dout = nc.dram_tensor("xslab", [N, S], F32, kind="ExternalOutput")

    with tile.TileContext(nc) as tc:
        with tc.tile_pool(name="main", bufs=1) as pool, \
             tc.tile_pool(name="io", bufs=4) as iopool, \
             tc.tile_pool(name="ps", bufs=8, space="PSUM") as pspool:
            lhs = pool.tile([4, NT * TB], F32R, tag="lhs")
            nc.sync.dma_start(lhs[:], dins["lhsT"][:])
            rhsb = pool.tile([4, NT * S], F32R, tag="rhsb")
            nc.sync.dma_start(rhsb[:], dins["rhs"][:])
            dg = pool.tile([TB, 4 * TB], F32, tag="dg")
            nc.sync.dma_start(dg[:], dins["diag"][:])
            for t in range(NT):
                ps = pspool.tile([TB, S], F32, tag="ps")
                nc.tensor.matmul(
                    ps[:],
                    lhs[:, t*TB:(t+1)*TB],
                    rhsb[:, t*S:(t+1)*S],
                )
                ob = iopool.tile([TB, S], F32, tag="ob")
                if t % 2 == 0:
                    nc.scalar.copy(ob[:], ps[:])
                    if t < 4:
                        nc.scalar.copy(ob[:, t*TB:(t+1)*TB], dg[:, t*TB:(t+1)*TB])
                else:
                    nc.vector.tensor_copy(ob[:], ps[:])
                    if t < 4:
                        nc.vector.tensor_copy(ob[:, t*TB:(t+1)*TB], dg[:, t*TB:(t+1)*TB])
                nc.sync.dma_start(dout[t*TB:(t+1)*TB, :], ob[:])

    # --- post-pass: this walrus build allows only 1 sync-wait per
    # instruction; split extras onto preceding same-engine NOPs ---
    def _split_waits(maxw=1):
        all_bbs = list(nc.main_func.blocks)
        for bb in all_bbs:
            out = []
            for inst in bb.instructions:
                si = getattr(inst, "sync_info", None)
                ow = list(si.on_wait) if (si is not None and si.on_wait) else []
                if len(ow) > maxw:
                    si.on_wait = ow[-maxw:]
                    try:
                        eng_builder = nc.engines[inst.engine]
                    except Exception:
                        eng_builder = nc.sync
                    for w in ow[:-maxw]:
                        nop = eng_builder.nop()
                        for bb2 in nc.main_func.blocks:
                            li = bb2.instructions
                            if li and li[-1] is nop.ins:
                                li.pop()
                                break
                        nop.ins.sync_info = mybir.SyncInfo(on_wait=[w], on_update=[])
                        out.append(nop.ins)
                out.append(inst)
            bb.instructions[:] = out
    _split_waits()
    return nc, dins, dout


def _device_run(P):
    from concourse.bass_utils import run_bass_kernel_spmd
    if "nc" not in _CACHED:
        _CACHED["nc"] = _build_nc()
    nc, dins, dout = _CACHED["nc"]
    in_maps = [_core_inputs(P, core) for core in range(NCORES)]
    res = run_bass_kernel_spmd(nc, in_maps, list(range(NCORES)))
    X = np.zeros((N, N), np.float32)
    for core in range(NCORES):
        slab = res.results[core]["xslab"]
        order = _tile_order(core)
        for t, g in enumerate(order):
            X[TB*g:TB*(g+1), SLAB*core:SLAB*(core+1)] = slab[TB*t:TB*(t+1)]
    return X


def kernel(x, rho, sigma2):
    x = np.asarray(x, dtype=np.float64)
    rho = float(np.asarray(rho)); sigma2 = float(np.asarray(sigma2))
    Bcols = _stage1_bands(x, rho, sigma2)
    P = _host_pieces(Bcols)
    _CACHED["P_obj"] = P
    X = _device_run(P).astype(np.float64)
    return X


# revision 9
# speedup vs baseline: 7.3061x; 1.5293x over previous
"""Trainium2 kernel: X = inv(phi + sigma2*A) for the DeepKernelPacketGP module.

Math: B = phi + sigma2*A is exactly pentadiagonal, so X = inv(B) is
semiseparable: X[i,j] = F[i,:] @ C[:,j] for i < j (rank 2), G[i,:] @ D[:,j]
for i > j, plus the diagonal. Host (f64, O(n)): banded solves for F, G
(columns 0,1 and n-2,n-1 of X), batched local 5x5 solves per column for
C, D, diag. Device (8 cores, column-slab sharding): per 128x512 output
tile one K=4 fp32r matmul (per-row-block QR-orthonormalized generators),
PSUM->SBUF copy, DMA out. Diagonal 128x128 blocks are host-precomputed in
f64 and overwritten after the matmul; each core processes its 4 diagonal
tiles first so the overwrite offset is core-invariant (SPMD), and the host
unscrambles the row order on gather.
"""
import sys
sys.path.insert(0, '/opt/trn_rl_repo')
import numpy as np
from scipy.linalg import solve_banded

N = 4096
NCORES = 8
SLAB = N // NCORES         # 512
TB = 128                   # tile rows
NT = N // TB               # 32 tiles per slab

# ============================================================================
# Host math (float64)
# ============================================================================

def _stage1_bands(x, rho, sigma2):
    n = x.shape[0]; k = 5; m = 2; n_pow = 2
    c = np.sqrt(3.0) / rho
    W = n - 4
    idx = np.arange(W)[:, None] + np.arange(k)[None, :]
    xw = x[idx]
    t = xw - (xw[:, :1] + xw[:, -1:]) / 2
    pw = t[:, :, None] ** np.arange(n_pow)
    pos = pw * np.exp(c * t)[:, :, None]
    neg = pw * np.exp(-c * t)[:, :, None]
    e_first = np.zeros((W, 1, k)); e_first[:, :, 0] = 1.0
    Amat = np.concatenate([np.swapaxes(pos, 1, 2), np.swapaxes(neg, 1, 2), e_first], axis=1)
    rhs = np.zeros((k,)); rhs[-1] = 1.0
    a = np.linalg.solve(Amat, np.broadcast_to(rhs, (W, k))[..., None])[..., 0]
    d = np.abs(xw[:, :, None] - xw[:, None, :]); s = c * d
    Kw = (1 + s) * np.exp(-s)
    phiv = np.einsum('wij,wj->wi', Kw, a)
    bcol = phiv + sigma2 * a
    Bcols = np.zeros((n, 5))
    Bcols[2:n-2, :] = bcol
    def bnd(xseg, tshift, npos, nneg):
        ss = xseg.shape[0]
        xt = xseg + tshift
        rows = [xt**j * np.exp(c*xt) for j in range(npos)]
        rows += [xt**j * np.exp(-c*xt) for j in range(nneg)]
        e = np.zeros(ss); e[0] = 1.0
        rows.append(e)
        M = np.stack(rows); r = np.zeros(ss); r[-1] = 1.0
        aa = np.linalg.solve(M, r)
        dd = np.abs(xseg[:, None] - xseg[None, :]); s2 = c*dd
        return aa, ((1+s2)*np.exp(-s2)) @ aa
    for i in range(m):
        s_l = i + m + 1
        aa, pp = bnd(x[:s_l], -x[s_l-1], n_pow, s_l - 3)
        for r in range(s_l):
            Bcols[i, r - i + 2] = pp[r] + sigma2*aa[r]
        s_r = k - 1 - i
        aa, pp = bnd(x[n-s_r:], -x[n-s_r], s_r - 3, n_pow)
        col = n - m + i
        for ridx in range(s_r):
            r = n - s_r + ridx
            Bcols[col, r - col + 2] = pp[ridx] + sigma2*aa[ridx]
    return Bcols


def _host_generators(Bcols):
    """F, G (n x 2), C, D (2 x n), xd (n): semiseparable generators of inv(B)."""
    n = N
    ab = Bcols.T.copy()              # scipy banded form: ab[2+d, c] = B[c+d, c]
    E = np.zeros((n, 4))
    E[n-2, 0] = 1; E[n-1, 1] = 1; E[0, 2] = 1; E[1, 3] = 1
    sol = solve_banded((2, 2), ab, E)
    F = sol[:, 0:2]; G = sol[:, 2:4]

    Brow = np.zeros((5, n))          # Brow[2+d, j] = B[j, j+d]
    for d in range(-2, 3):
        jj = np.arange(max(0, -d), n - max(0, d))
        Brow[2 + d, jj] = Bcols[jj + d, 2 - d]

    def BofRI(r, i):
        d = i - r
        out = np.zeros(r.shape)
        ok = (np.abs(d) <= 2) & (r >= 0) & (r < n) & (i >= 0) & (i < n)
        out[ok] = Brow[2 + d[ok], r[ok]]
        return out

    # interior columns j=2..n-3: 5x5 solve for [c1, c2, xjj, d1, d2]
    jj = np.arange(2, n - 2)
    M = np.zeros((jj.size, 5, 5))
    for r_loc in range(5):
        r = jj - 2 + r_loc
        for t in range(-2, 3):
            i = r + t
            b = BofRI(r, i)
            ic = np.clip(i, 0, n - 1)
            lo = i < jj; eq = i == jj; hi = i > jj
            M[lo, r_loc, 0] += b[lo] * F[ic[lo], 0]
            M[lo, r_loc, 1] += b[lo] * F[ic[lo], 1]
            M[eq, r_loc, 2] += b[eq]
            M[hi, r_loc, 3] += b[hi] * G[ic[hi], 0]
            M[hi, r_loc, 4] += b[hi] * G[ic[hi], 1]
    rhs = np.zeros((jj.size, 5)); rhs[:, 2] = 1.0
    U = np.linalg.solve(M, rhs[..., None])[..., 0]
    C = np.zeros((2, n)); D = np.zeros((2, n)); xd = np.zeros(n)
    C[:, jj] = U[:, 0:2].T
    xd[jj] = U[:, 2]
    D[:, jj] = U[:, 3:5].T

    # boundary columns
    M3 = np.zeros((3, 3)); r3 = np.arange(3)
    for t in range(-2, 3):
        i = r3 + t; b = BofRI(r3, i); ic = np.clip(i, 0, n - 1)
        M3[:, 0] += b * (i == 0)
        for k in range(2):
            M3[:, 1 + k] += b * G[ic, k] * (i > 0)
    u = np.linalg.solve(M3, np.eye(3)[0])
    xd[0] = u[0]; D[:, 0] = u[1:3]

    M4 = np.zeros((4, 4)); r4 = np.arange(4)
    for t in range(-2, 3):
        i = r4 + t; b = BofRI(r4, i); ic = np.clip(i, 0, n - 1)
        M4[:, 0] += b * (i == 0)
        M4[:, 1] += b * (i == 1)
        for k in range(2):
            M4[:, 2 + k] += b * G[ic, k] * (i > 1)
    u = np.linalg.solve(M4, np.eye(4)[1])
    xd[1] = u[1]; D[:, 1] = u[2:4]
    # X[0,1] needed for the (0,0) diagonal center block
    x01 = u[0]

    M4 = np.zeros((4, 4)); r4 = np.arange(n - 4, n)
    for t in range(-2, 3):
        i = r4 + t; b = BofRI(r4, i); ic = np.clip(i, 0, n - 1)
        for k in range(2):
            M4[:, k] += b * F[ic, k] * (i < n - 2)
        M4[:, 2] += b * (i == n - 2)
        M4[:, 3] += b * (i == n - 1)
    u = np.linalg.solve(M4, np.eye(4)[2])
    C[:, n - 2] = u[0:2]; xd[n - 2] = u[2]
    xn12 = u[3]   # X[n-1, n-2]

    M3 = np.zeros((3, 3)); r3 = np.arange(n - 3, n)
    for t in range(-2, 3):
        i = r3 + t; b = BofRI(r3, i); ic = np.clip(i, 0, n - 1)
        for k in range(2):
            M3[:, k] += b * F[ic, k] * (i < n - 1)
        M3[:, 2] += b * (i == n - 1)
    u = np.linalg.solve(M3, np.eye(3)[2])
    C[:, n - 1] = u[0:2]; xd[n - 1] = u[2]

    return F, G, C, D, xd, x01, xn12


def _host_pieces(Bcols):
    F, G, C, D, xd, x01, xn12 = _host_generators(Bcols)
    n = N
    # per-row-block QR bases
    Qf = np.zeros((NT, TB, 2)); Rf = np.zeros((NT, 2, 2))
    Qg = np.zeros((NT, TB, 2)); Rg = np.zeros((NT, 2, 2))
    for g in range(NT):
        rows = slice(TB * g, TB * (g + 1))
        Qf[g], Rf[g] = np.linalg.qr(F[rows])
        Qg[g], Rg[g] = np.linalg.qr(G[rows])
    # diagonal center blocks (f64)
    cen = np.zeros((NT, TB, TB))
    ii = np.arange(TB)[:, None]; jc = np.arange(TB)[None, :]
    for g in range(NT):
        rows = slice(TB * g, TB * (g + 1))
        up = F[rows] @ C[:, rows]
        loP = G[rows] @ D[:, rows]
        blk = np.where(ii < jc, up, np.where(ii > jc, loP, 0.0))
        blk[np.arange(TB), np.arange(TB)] = xd[TB * g:TB * (g + 1)]
        cen[g] = blk
    # exact corner values not covered by the rank-2 reps
    cen[0][0, 1] = x01
    cen[NT - 1][TB - 1, TB - 2] = xn12
    return dict(F=F, G=G, C=C, D=D, xd=xd, Qf=Qf, Rf=Rf, Qg=Qg, Rg=Rg, cen=cen)


def _tile_order(core):
    """Processing order of row-blocks for this core: its 4 diag blocks first."""
    own = [4 * core + k for k in range(4)]
    rest = [g for g in range(NT) if g // 4 != core]
    return own + rest


def _core_inputs(P, core):
    from ml_dtypes import bfloat16
    cols = slice(SLAB * core, SLAB * (core + 1))
    order = _tile_order(core)
    lhsT = np.zeros((NT, 4, TB), np.float64)
    rhs = np.zeros((NT, 4, SLAB), np.float64)
    colblk = (np.arange(SLAB * core, SLAB * (core + 1)) // TB)  # global 128-block
    for t, g in enumerate(order):
        lhsT[t, 0:2] = P["Qf"][g].T
        lhsT[t, 2:4] = P["Qg"][g].T
        mC = (colblk > g).astype(np.float64)
        mD = (colblk < g).astype(np.float64)
        rhs[t, 0:2] = (P["Rf"][g] @ P["C"][:, cols]) * mC
        rhs[t, 2:4] = (P["Rg"][g] @ P["D"][:, cols]) * mD
    diag = np.zeros((TB, 4 * TB), np.float32)
    for k in range(4):
        diag[:, k*TB:(k+1)*TB] = P["cen"][4 * core + k]
    return {"lhsT": lhsT.astype(bfloat16), "rhs": rhs.astype(bfloat16),
            "diag": diag}


# ============================================================================
# Device kernel
# ============================================================================

_CACHED = {}

def _build_nc():
    import concourse.bass as bass
    import concourse.mybir as mybir
    import concourse.tile as tile
    from concourse.vector_clock import ScopedClock

    def _patched_drain_and_barrier(self, tick_clock, wait_clock):
        nopw = self.nc.gpsimd.nop()
        wait_clock.add_sem_waits(nopw.ins, ScopedClock({None: tick_clock.global_clock}))
        waits = list(nopw.ins.sync_info.on_wait) if nopw.ins.sync_info else []
        if len(waits) > 1:
            nopw.ins.sync_info.on_wait = waits[:1]
            for w in waits[1:]:
                extra = self.nc.gpsimd.nop()
                extra.ins.sync_info = mybir.SyncInfo(on_wait=[w], on_update=[])
        self.nc.sync.drain()
        self.nc.all_engine_barrier()
        assert self.sems is not None
        popped = self.nc._tile_sem_poison_stack.pop()
        assert popped is self._sem_poison
        self.nc.clear_and_free_semaphores(list(self.sems.allocated().values()))
        self.nc.all_engine_barrier()
    tile.TileContext._drain_and_barrier = _patched_drain_and_barrier

    F32 = mybir.dt.float32
    BF16 = mybir.dt.bfloat16
    S = SLAB
    AP = bass.AP

    nc = bass.Bass(target_bir_lowering=False)
    dins = {
        "lhsT": nc.dram_tensor("lhsT", [NT, 4, TB], BF16, kind="ExternalInput"),
        "rhs": nc.dram_tensor("rhs", [NT, 4, S], BF16, kind="ExternalInput"),
        "diag": nc.dram_tensor("diag", [TB, 4 * TB], F32, kind="ExternalInput"),
    }
    dout = nc.dram_tensor("xslab", [N, S], BF16, kind="ExternalOutput")

    with tile.TileContext(nc) as tc:
        with tc.tile_pool(name="main", bufs=1) as pool, \
             tc.tile_pool(name="io", bufs=3) as iopool, \
             tc.tile_pool(name="ps", bufs=8, space="PSUM") as pspool:
            # inputs: t-major DRAM layout so each partition's data is many
            # short runs -> packets spread across all DMA engines
            lhs = pool.tile([4, NT * TB], BF16, tag="lhs")
            src = dins["lhsT"][:]
            nc.gpsimd.dma_start(
                AP(lhs[:].tensor, lhs[:].offset,
                   [[NT * TB, 4], [TB, NT], [1, TB]]),
                AP(src.tensor, src.offset, [[TB, 4], [4 * TB, NT], [1, TB]]))
            rhsb = pool.tile([4, NT * S], BF16, tag="rhsb")
            src = dins["rhs"][:]
            nc.gpsimd.dma_start(
                AP(rhsb[:].tensor, rhsb[:].offset,
                   [[NT * S, 4], [S, NT], [1, S]]),
                AP(src.tensor, src.offset, [[S, 4], [4 * S, NT], [1, S]]))
            dg = pool.tile([TB, 4 * TB], F32, tag="dg")
            nc.scalar.dma_start(dg[:], dins["diag"][:])
            for grp in range(NT // 4):
                ob = iopool.tile([TB, 4 * S], BF16, tag="ob")
                for k in range(4):
                    t = 4 * grp + k
                    ps = pspool.tile([TB, S], F32, tag="ps")
                    nc.tensor.matmul(
                        ps[:],
                        lhs[:, t*TB:(t+1)*TB],
                        rhsb[:, t*S:(t+1)*S],
                    )
                    dst = ob[:, k*S:(k+1)*S]
                    if k % 2 == 0:
                        nc.scalar.copy(dst, ps[:])
                        if grp == 0:
                            nc.scalar.copy(ob[:, k*S + t*TB:k*S + (t+1)*TB],
                                           dg[:, t*TB:(t+1)*TB])
                    else:
                        nc.vector.tensor_copy(dst, ps[:])
                        if grp == 0:
                            nc.vector.tensor_copy(ob[:, k*S + t*TB:k*S + (t+1)*TB],
                                                  dg[:, t*TB:(t+1)*TB])
                # one DMA for 4 tiles: dram rows [512*grp, 512*(grp+1))
                ob_ap = ob[:]
                dout_ap = dout[:]
                nc.sync.dma_start(
                    AP(dout_ap.tensor, 4 * grp * TB * S,
                       [[S, TB], [TB * S, 4], [1, S]]),
                    AP(ob_ap.tensor, ob_ap.offset,
                       [[4 * S, TB], [S, 4], [1, S]]))

    # --- post-pass: this walrus build allows only 1 sync-wait per
    # instruction; split extras onto preceding same-engine NOPs ---
    def _split_waits(maxw=1):
        all_bbs = list(nc.main_func.blocks)
        for bb in all_bbs:
            out = []
            for inst in bb.instructions:
                si = getattr(inst, "sync_info", None)
                ow = list(si.on_wait) if (si is not None and si.on_wait) else []
                if len(ow) > maxw:
                    si.on_wait = ow[-maxw:]
                    try:
                        eng_builder = nc.engines[inst.engine]
                    except Exception:
                        eng_builder = nc.sync
                    for w in ow[:-maxw]:
                        nop = eng_builder.nop()
                        for bb2 in nc.main_func.blocks:
                            li = bb2.instructions
                            if li and li[-1] is nop.ins:
                                li.pop()
                                break
                        nop.ins.sync_info = mybir.SyncInfo(on_wait=[w], on_update=[])
                        out.append(nop.ins)
                out.append(inst)
            bb.instructions[:] = out
    _split_waits()
    return nc, dins, dout


def _device_run(P):
    from concourse.bass_utils import run_bass_kernel_spmd
    if "nc" not in _CACHED:
        _CACHED["nc"] = _build_nc()
    nc, dins, dout = _CACHED["nc"]
    in_maps = [_core_inputs(P, core) for core in range(NCORES)]
    res = run_bass_kernel_spmd(nc, in_maps, list(range(NCORES)))
    X = np.zeros((N, N), np.float32)
    for core in range(NCORES):
        slab = res.results[core]["xslab"]
        order = _tile_order(core)
        for t, g in enumerate(order):
            X[TB*g:TB*(g+1), SLAB*core:SLAB*(core+1)] = slab[TB*t:TB*(t+1)]
    return X


def kernel(x, rho, sigma2):
    x = np.asarray(x, dtype=np.float64)
    rho = float(np.asarray(rho)); sigma2 = float(np.asarray(sigma2))
    Bcols = _stage1_bands(x, rho, sigma2)
    P = _host_pieces(Bcols)
    _CACHED["P_obj"] = P
    X = _device_run(P).astype(np.float64)
    return X
